# revision 17
# baseline (speedup 1.0000x reference)
"""Multi-head attention (B=4, S=2048, D=1024, H=16, Dh=64) on 8 trn2 NeuronCores.

Sharding: core c -> heads (2c, 2c+1) of ALL 4 batches.  Every batch has 16
heads, so each core gets exactly 2 heads x 4 batches and per-core attention
work is Sum_b SQT_b*SKT_b score tiles -- perfectly balanced across cores
regardless of the per-batch sequence lengths (the old batch-sharded layout
made the largest-batch core ~2.1x slower than the mean).

Per core (2 heads, head A on partitions 0:64, head B on 64:128):
  - Host pre-transposes X per batch (D-major) in bf16 and concatenates the
    batches along seq: xq [D, SQtot], xk/xv [D, SKtot] (V rows >= V_len are
    zeroed on host).
  - Projections: qT/kT in [dh, seq] orientation, v in natural [seq, dh]
    orientation with a mask column appended per head (denominator trick).
  - QK computes scoresT[sk, sq] with K=64 contraction, the two heads issued
    back-to-back to complementary row groups (tile_position (0,0)/(64,0)) so
    they run concurrently in the PE array -- 2x QK throughput vs zero-padding
    the contraction to 128.
  - exp on ScalarE in groups of up to 3 sk-tiles (one 3-bank PSUM tile per
    group) to amortize the ~293ns fixed ACTIVATE overhead.
  - PV accumulates oT[65, sq] per head (row 64 = softmax denominator via the
    mask column).  NO on-device transpose or normalization: the kernel ships
    oT + denominator to DRAM and the HOST does o = (num/den).T and the
    Q_len row masking during unsharding.  This removes the fp32 PE-transpose
    matmuls (~80us of PE time in the old kernel) entirely.
  - Emission is software-pipelined: the next batch's projection pieces and
    the previous chunk's PV pieces are interleaved between QK groups to keep
    the in-order PE queue dense while ScalarE (the attention-phase
    bottleneck) drains the exp queue.

The program is compiled for the runtime tile counts (SQT_b, SKT_b) =
ceil(len/128) per batch (shared SPMD program across the 8 cores).
"""

import math

import numpy as np
import ml_dtypes


def _ensure_paths():
    import sys
    try:
        import concourse  # noqa: F401
        return
    except ImportError:
        pass
    for p in ("/opt/trn_rl_repo", "/root/.axon_site/_ro/trn_rl_repo"):
        if p not in sys.path:
            sys.path.insert(0, p)
    import concourse  # noqa: F401


P = 128          # SBUF partitions
D = 1024         # model dim
DH = 64          # head dim
KT = D // P      # contraction tiles for projections
GN = 3           # sk-tiles per exp group (3 PSUM banks)
NB = 4           # batches
NCORES = 8

_PROG_CACHE = {}

# exposed for test.py profiling reruns
_last_nc = None
_last_in_maps = None


def _chunks(total, sz=512):
    out = []
    o = 0
    while o < total:
        n = min(sz, total - o)
        out.append((o, n))
        o += n
    return out


def _build_program(SQT, SKT):
    """Build + bacc-compile the shared SPMD program for given per-batch tile
    counts (SQT, SKT are 4-tuples)."""
    _ensure_paths()
    import concourse.bass as bass  # noqa: F401
    import concourse.tile as tile
    from concourse import bacc, mybir

    BF = mybir.dt.bfloat16
    F32 = mybir.dt.float32
    Exp = mybir.ActivationFunctionType.Exp

    SQ = [t * P for t in SQT]
    SK = [t * P for t in SKT]
    SQtot = sum(SQ)
    SKtot = sum(SK)
    QOFF = [sum(SQ[:b]) for b in range(NB)]
    KOFF = [sum(SK[:b]) for b in range(NB)]
    TOFF = [sum(SKT[:b]) for b in range(NB)]
    SKTtot = sum(SKT)
    ATM = max(SKT)

    nc = bacc.Bacc("TRN2", target_bir_lowering=False, debug=False,
                   num_devices=NCORES)

    xq = nc.dram_tensor("xq", [D, SQtot], BF, kind="ExternalInput").ap()
    xk = nc.dram_tensor("xk", [D, SKtot], BF, kind="ExternalInput").ap()
    xv = nc.dram_tensor("xv", [D, SKtot], BF, kind="ExternalInput").ap()
    wq = nc.dram_tensor("wq", [D, P], BF, kind="ExternalInput").ap()
    wk = nc.dram_tensor("wk", [D, P], BF, kind="ExternalInput").ap()
    wv = nc.dram_tensor("wv", [D, P], BF, kind="ExternalInput").ap()
    mk2 = nc.dram_tensor("mk2", [P, SKTtot, 2], BF, kind="ExternalInput").ap()
    out = nc.dram_tensor("out", [2, DH + 1, SQtot], F32,
                         kind="ExternalOutput").ap()

    xq_r = xq.rearrange("(k p) s -> p k s", p=P)
    xk_r = xk.rearrange("(k p) s -> p k s", p=P)
    xv_r = xv.rearrange("(k p) s -> p k s", p=P)

    VW = 2 * (DH + 1)        # 130: [A num 64 | A mask | B num 64 | B mask]

    with tile.TileContext(nc) as tc:
        with tc.tile_pool(name="const", bufs=1) as const, \
             tc.tile_pool(name="persist", bufs=1) as persist, \
             tc.tile_pool(name="xs", bufs=10) as xs, \
             tc.tile_pool(name="atp", bufs=3) as atp, \
             tc.tile_pool(name="otp", bufs=2) as otp, \
             tc.tile_pool(name="psq", bufs=2, space="PSUM") as psq, \
             tc.tile_pool(name="pss", bufs=2, space="PSUM") as pss:

            wq_sb = const.tile([P, KT, P], BF, tag="wq")
            wk_sb = const.tile([P, KT, P], BF, tag="wk")
            wv_sb = const.tile([P, KT, P], BF, tag="wv")
            mask_sb = const.tile([P, SKTtot, 2], BF, tag="mk")
            # wk on the sync queue ahead of the x stream (first QK group
            # needs it); wq/wv/mask via gpsimd so xk0 starts 0.8us earlier
            nc.sync.dma_start(out=wk_sb, in_=wk.rearrange("(k p) e -> p k e", p=P))
            nc.gpsimd.dma_start(out=wq_sb, in_=wq.rearrange("(k p) e -> p k e", p=P))
            nc.gpsimd.dma_start(out=wv_sb, in_=wv.rearrange("(k p) e -> p k e", p=P))
            nc.gpsimd.dma_start(out=mask_sb, in_=mk2)

            qt = persist.tile([P, SQtot], BF, tag="qt")
            kt = persist.tile([P, SKtot], BF, tag="kt")
            v_sb = persist.tile([P, SKTtot, VW], BF, tag="v")

            # PE warm-up: ~11us of dummy matmuls with no input dependencies.
            # They run during the initial input-DMA wait and flip the HAM
            # clock gate to 8/8 so the first real matmuls run at 2.4 GHz.
            warm = const.tile([P, 512], BF, tag="warm")
            nc.gpsimd.memset(warm, 0.0)
            for w in range(20):
                wp = pss.tile([P, 512], F32, tag="acc", name=f"warm_{w}")
                nc.tensor.matmul(wp, warm[:, 0:P], warm,
                                 start=True, stop=True)

            # ---------------- input-DMA pump + mm pieces ----------------
            # The x-input DMAs are DECOUPLED from the matmul pieces: all
            # transfers are emitted in need-order on the ONE sync queue,
            # ~XS_DEPTH pieces ahead of their consumers (the xs ring
            # throttles at runtime).  The queue then never idles, which
            # matters because the input stream (~26 MB at ~330 GB/s) has
            # almost no slack against the compute schedule.
            XS_DEPTH = 10
            dma_stream = []      # need-ordered [(key, emit_fn)]
            dma_emitted = {}     # key -> xt tile (set when DMA emitted)
            mm_done_idx = set()  # stream indices whose mm part has run
            pump_state = [0]     # next stream index to emit

            def _mk_dma(key, src_r, off, c0, n):
                def emit():
                    if key in dma_emitted:
                        return
                    xt = xs.tile([P, KT, 512], BF, tag="x")
                    nc.sync.dma_start(out=xt[:, :, :n],
                                      in_=src_r[:, :, off + c0:off + c0 + n])
                    dma_emitted[key] = xt
                return (key, emit)

            def pump_dma():
                """Emit pending input DMAs: stream index i may be emitted
                once the reader of ring slot i-XS_DEPTH has been emitted
                (pool WAR dep must point backward in program order)."""
                while pump_state[0] < len(dma_stream):
                    i = pump_state[0]
                    if i >= XS_DEPTH and i - XS_DEPTH not in mm_done_idx:
                        return
                    dma_stream[i][1]()
                    pump_state[0] = i + 1

            stream_idx = {}      # key -> position in dma_stream

            def q_proj_piece(b, c0, n, dst, w_sb, off, kind):
                key = (kind, b, c0)

                def go():
                    if key not in dma_emitted:       # safety: force-emit
                        dma_stream[stream_idx[key]][1]()
                    xt = dma_emitted[key]
                    ps = pss.tile([P, 512], F32, tag="acc")
                    for k in range(KT):
                        nc.tensor.matmul(ps[:, :n], w_sb[:, k, :], xt[:, k, :n],
                                         start=(k == 0), stop=(k == KT - 1))
                    nc.vector.tensor_copy(out=dst[:, off + c0:off + c0 + n],
                                          in_=ps[:, :n])
                    mm_done_idx.add(stream_idx[key])
                    pump_dma()
                return go

            def v_proj_piece(b, c0, n):
                key = ('v', b, c0)

                def go():
                    if key not in dma_emitted:       # safety: force-emit
                        dma_stream[stream_idx[key]][1]()
                    xt = dma_emitted[key]
                    nt = n // P
                    ta = TOFF[b] + c0 // P
                    for st in range(nt):
                        ps = pss.tile([P, 512], F32, tag="acc")
                        for k in range(KT):
                            nc.tensor.matmul(
                                ps[:, :P], xt[:, k, P * st:P * (st + 1)],
                                wv_sb[:, k, :],
                                start=(k == 0), stop=(k == KT - 1))
                        vt = v_sb[:, ta + st, 0:VW].rearrange(
                            "p (g c) -> p g c", c=DH + 1)
                        nc.vector.tensor_copy(
                            out=vt[:, :, 0:DH],
                            in_=ps[:, :P].rearrange("p (g c) -> p g c", c=DH))
                        nc.vector.tensor_copy(
                            out=vt[:, :, DH:DH + 1],
                            in_=mask_sb[:, ta + st:ta + st + 1, :].rearrange(
                                "p t g -> p g t"))
                    mm_done_idx.add(stream_idx[key])
                    pump_dma()
                return go

            def q_pieces(b):
                return [q_proj_piece(b, c0, n, qt, wq_sb, QOFF[b], 'q')
                        for c0, n in chl[b]]

            def prelude_a(b):
                """What batch b's first QK group needs: first kT chunk + the
                first q chunk."""
                kp = [q_proj_piece(b, c0, n, kt, wk_sb, KOFF[b], 'k')
                      for c0, n in kchl[b]]
                return kp[:1] + [q_pieces(b)[0]]

            def prelude_rest(b):
                return [q_proj_piece(b, c0, n, kt, wk_sb, KOFF[b], 'k')
                        for c0, n in kchl[b]][1:]

            def prelude_b(b):
                """v projection -- only needed by PV, one chunk after QK."""
                return [v_proj_piece(b, c0, n) for c0, n in _chunks(SK[b])]

            # ---------------- attention ----------------
            def pv_pieces(b, c0, n, ats, tail_dma=False):
                """PV + evac + output-DMA closures for one finished chunk."""
                pieces = []

                def mk_pv(g, t0, t1, po_box):
                    def go():
                        if t0 == 0:
                            po_box[0] = pss.tile([P, 512], F32, tag="acc",
                                                 name=f"po_{b}_{c0}_{g}")
                        po = po_box[0]
                        for t in range(t0, t1):
                            # lhsT is only the 65 live columns: LDWEIGHTS
                            # time scales with column count
                            nc.tensor.matmul(
                                po[0:DH + 1, :n],
                                v_sb[:, TOFF[b] + t, (DH + 1) * g:
                                     (DH + 1) * (g + 1)],
                                ats[g][:, t, :n],
                                start=(t == 0), stop=(t == SKT[b] - 1))
                        if t1 == SKT[b]:
                            ot = otp.tile([DH + 1, 512], F32, tag="ot",
                                          name=f"ot_{b}_{c0}_{g}")
                            nc.vector.tensor_copy(out=ot[:, :n],
                                                  in_=po[0:DH + 1, :n])
                            # output DMA on the (idle) GpSimd queue so it
                            # never blocks input prefetch on the sync queue;
                            # the tail chunks go via the scalar queue so the
                            # final flush drains two queues in parallel
                            eng = nc.scalar if tail_dma else nc.gpsimd
                            eng.dma_start(
                                out=out[g, :, QOFF[b] + c0:QOFF[b] + c0 + n],
                                in_=ot[:, :n])
                    return go

                for g in range(2):
                    box = [None]
                    if SKT[b] > 1:
                        half = (SKT[b] + 1) // 2
                        pieces.append(mk_pv(g, 0, half, box))
                        pieces.append(mk_pv(g, half, SKT[b], box))
                    else:
                        pieces.append(mk_pv(g, 0, SKT[b], box))
                return pieces

            # PV pieces of chunk c are emitted two chunks later (the at
            # ring is 3 deep), so side-work DMA has two chunks of exp time
            # to stream in before it can stall the PE queue
            pendq = []         # FIFO of per-chunk PV piece lists

            def emit_chunk(b, c0, n, fresh, tail=False):
                """Emit one attention chunk; `fresh` (proj pieces) and the
                due PV pieces are interleaved between QK groups."""
                ats = (atp.tile([P, ATM, 512], BF, tag="ata",
                                name=f"ata_{b}_{c0}"),
                       atp.tile([P, ATM, 512], BF, tag="atb",
                                name=f"atb_{b}_{c0}"))
                groups = [(t0, min(GN, SKT[b] - t0))
                          for t0 in range(0, SKT[b], GN)]
                side = list(fresh)
                if len(pendq) >= 1:
                    side = side + pendq.pop(0)
                L = len(side)
                done = 0
                for gi, (t0, gn) in enumerate(groups):
                    pq = [psq.tile([P, GN, 512], F32, tag="qk",
                                   name=f"qk_{b}_{c0}_{t0}_{g}")
                          for g in range(2)]
                    for j in range(gn):
                        t = t0 + j
                        for g in range(2):
                            nc.tensor.matmul(
                                pq[g][:, j, :n],
                                kt[DH * g:DH * (g + 1),
                                   KOFF[b] + P * t:KOFF[b] + P * (t + 1)],
                                qt[DH * g:DH * (g + 1),
                                   QOFF[b] + c0:QOFF[b] + c0 + n],
                                start=True, stop=True)
                    for g in range(2):
                        nc.scalar.activation(
                            out=ats[g][:, t0:t0 + gn, :n],
                            in_=pq[g][:, 0:gn, :n],
                            func=Exp, scale=0.125)
                    upto = (L * (gi + 1)) // len(groups)
                    while done < upto:
                        side[done]()
                        done += 1
                while done < L:
                    side[done]()
                    done += 1
                pendq.append(pv_pieces(b, c0, n, ats, tail))

            # process batches LARGEST-first: the big batch's long exp phase
            # covers the later batches' input DMA, and only kT + q0 of the
            # first batch gate the start of attention (v streams during the
            # first chunk, the rest of q during later chunks)
            BORD = sorted(range(NB), key=lambda b: -SQT[b] * SKT[b])

            # attention / q-proj chunk lists: a smaller (384) first chunk
            # for the first batch (less DMA before the first exp) and a
            # small final chunk for the last batch (shorter PV tail)
            chl = {b: _chunks(SQ[b]) for b in range(NB)}
            kchl = {b: _chunks(SK[b]) for b in range(NB)}
            if SQ[BORD[0]] > 512:
                chl[BORD[0]] = [(0, 256)] + \
                    [(256 + o, n) for o, n in _chunks(SQ[BORD[0]] - 256)]
            if SK[BORD[0]] > 512:
                kchl[BORD[0]] = [(0, 384)] + \
                    [(384 + o, n) for o, n in _chunks(SK[BORD[0]] - 384)]
            _lo, _ln = chl[BORD[-1]][-1]
            if _ln > 128:
                chl[BORD[-1]] = chl[BORD[-1]][:-1] + \
                    [(_lo, _ln - 128), (_lo + _ln - 128, 128)]

            # need-ordered input stream: per batch all k chunks, the first
            # two q chunks, then v interleaved with the remaining q
            for b in BORD:
                ent = [('k', b, c0, n) for c0, n in kchl[b]]
                ent += [('q', b, c0, n) for c0, n in chl[b][:2]]
                vch = [('v', b, c0, n) for c0, n in _chunks(SK[b])]
                qch = [('q', b, c0, n) for c0, n in chl[b][2:]]
                inter = []
                for j in range(max(len(vch), len(qch))):
                    if j < len(vch):
                        inter.append(vch[j])
                    if j < len(qch):
                        inter.append(qch[j])
                for kind, bb, c0, n in ent + inter:
                    src, off = {'k': (xk_r, KOFF[bb]), 'q': (xq_r, QOFF[bb]),
                                'v': (xv_r, KOFF[bb])}[kind]
                    key = (kind, bb, c0)
                    stream_idx[key] = len(dma_stream)
                    dma_stream.append(_mk_dma(key, src, off, c0, n))
            pump_dma()           # emit the initial XS_DEPTH transfers

            for piece in prelude_a(BORD[0]):
                piece()
            carry_kt = prelude_rest(BORD[0])
            carry_v = prelude_b(BORD[0])
            for bi, b in enumerate(BORD):
                nxt = BORD[bi + 1] if bi + 1 < NB else None
                # carry_kt = this batch's remaining kT chunks: chunk 0's
                # later QK groups need them, so they lead chunk 0's side.
                # carry_v = its v pieces: must emit before PV of chunk 0
                # (which is emitted during chunk 2) -> spread over chunks
                # 0 and 1.
                kt0_, v0_ = carry_kt, carry_v
                filler = (prelude_a(nxt) + prelude_rest(nxt)) \
                    if nxt is not None else []
                carry_kt = []
                carry_v = prelude_b(nxt) if nxt is not None else []
                qr = q_pieces(b)[1:]
                ch = chl[b]
                vh = (len(v0_) + 1) // 2 if len(ch) > 1 else len(v0_)
                fdone = 0
                for ci, (c0, n) in enumerate(ch):
                    take = (len(filler) * (ci + 1)) // len(ch) - fdone
                    # q-proj pieces two chunks ahead of their QK consumer
                    if ci == 0:
                        qf = qr[0:2]
                    else:
                        qf = qr[ci + 1:ci + 2]
                    fresh = (kt0_ + v0_[:vh] if ci == 0 else []) \
                        + (v0_[vh:] if ci == 1 else []) \
                        + qf \
                        + filler[fdone:fdone + take]
                    fdone += take
                    emit_chunk(b, c0, n, fresh,
                               tail=(bi == NB - 1 and ci >= len(ch) - 2))
            for lst in pendq:
                for piece in lst:
                    piece()
            pendq.clear()

    nc.compile()
    return nc


def _get_program(SQT, SKT):
    key = (tuple(SQT), tuple(SKT))
    if key not in _PROG_CACHE:
        _PROG_CACHE[key] = _build_program(key[0], key[1])
    return _PROG_CACHE[key]


def _prep_inputs(Q_seq, K_seq, V_seq, WQ, WK, WV, Q_len, V_len):
    """Host-side shared prep: per-batch transposed bf16 activations and
    masks, concatenated along seq; returns (SQT, SKT, shared dict)."""
    BF = ml_dtypes.bfloat16
    B = Q_seq.shape[0]
    SQT = [max(1, math.ceil(int(Q_len[b]) / P)) for b in range(B)]
    SKT = [max(1, math.ceil(int(V_len[b]) / P)) for b in range(B)]
    SQ = [t * P for t in SQT]
    SK = [t * P for t in SKT]

    xq = np.concatenate(
        [np.ascontiguousarray(Q_seq[b, :SQ[b]].T) for b in range(B)],
        axis=1).astype(BF)
    xk = np.concatenate(
        [np.ascontiguousarray(K_seq[b, :SK[b]].T) for b in range(B)],
        axis=1).astype(BF)
    mks = [(np.arange(SK[b]) < int(V_len[b])) for b in range(B)]
    xv = np.concatenate(
        [np.ascontiguousarray((V_seq[b, :SK[b]] * mks[b][:, None]).T)
         for b in range(B)], axis=1).astype(BF)
    # mask laid out partition-major [128, SKTtot, 2] so the DMA moves
    # contiguous per-partition lines
    mkcat = np.concatenate(mks)                       # [SKtot]
    mk2 = np.repeat(
        mkcat.reshape(-1, P).T[:, :, None], 2, axis=2).astype(BF)
    return SQT, SKT, {"xq": xq, "xk": xk, "xv": xv, "mk2": mk2}


def kernel(Q_seq, K_seq, V_seq, WQ, WK, WV, Q_len, V_len):
    global _last_nc, _last_in_maps
    _ensure_paths()
    from concourse.bass_utils import run_bass_kernel_spmd

    Q_seq = np.asarray(Q_seq, dtype=np.float32)
    K_seq = np.asarray(K_seq, dtype=np.float32)
    V_seq = np.asarray(V_seq, dtype=np.float32)
    WQ = np.asarray(WQ, dtype=np.float32)
    WK = np.asarray(WK, dtype=np.float32)
    WV = np.asarray(WV, dtype=np.float32)
    Q_len = np.asarray(Q_len).reshape(-1)
    V_len = np.asarray(V_len).reshape(-1)

    B, S, _ = Q_seq.shape
    BF = ml_dtypes.bfloat16

    SQT, SKT, shared = _prep_inputs(Q_seq, K_seq, V_seq, WQ, WK, WV,
                                    Q_len, V_len)
    SQ = [t * P for t in SQT]
    QOFF = [sum(SQ[:b]) for b in range(B)]

    nc = _get_program(SQT, SKT)

    in_maps = []
    for c in range(NCORES):
        m = dict(shared)
        m["wq"] = np.ascontiguousarray(WQ[:, P * c:P * (c + 1)]).astype(BF)
        m["wk"] = np.ascontiguousarray(WK[:, P * c:P * (c + 1)]).astype(BF)
        m["wv"] = np.ascontiguousarray(WV[:, P * c:P * (c + 1)]).astype(BF)
        in_maps.append(m)

    res = run_bass_kernel_spmd(nc, in_maps, core_ids=list(range(NCORES)))
    _last_nc, _last_in_maps = nc, in_maps

    H = 16
    full = np.zeros((B, S, H * DH), dtype=np.float32)
    for c in range(NCORES):
        o = res.results[c]["out"]          # [2, 65, SQtot]
        for g in range(2):
            h = 2 * c + g
            num = o[g, :DH]                # [64, SQtot]
            den = o[g, DH:DH + 1]          # [1, SQtot]
            ot = num / den
            for b in range(B):
                ql = int(Q_len[b])
                sl = ot[:, QOFF[b]:QOFF[b] + SQ[b]]
                full[b, :SQ[b], DH * h:DH * (h + 1)] = sl.T
                full[b, ql:, DH * h:DH * (h + 1)] = 0.0
    return full



# revision 25
# speedup vs baseline: 1.2185x; 1.2185x over previous
"""Multi-head attention (B=4, S=2048, D=1024, H=16, Dh=64) on 8 trn2 NeuronCores.

Sharding: core c -> heads (2c, 2c+1) of ALL 4 batches.  Every batch has 16
heads, so each core gets exactly 2 heads x 4 batches and per-core attention
work is Sum_b SQT_b*SKT_b score tiles -- perfectly balanced across cores
regardless of the per-batch sequence lengths (the old batch-sharded layout
made the largest-batch core ~2.1x slower than the mean).

Per core (2 heads, head A on partitions 0:64, head B on 64:128):
  - Host pre-transposes X per batch (D-major) in bf16 and concatenates the
    batches along seq: xq [D, SQtot], xk/xv [D, SKtot] (V rows >= V_len are
    zeroed on host).
  - Projections: qT/kT in [dh, seq] orientation, v in natural [seq, dh]
    orientation with a mask column appended per head (denominator trick).
  - QK computes scoresT[sk, sq] with K=64 contraction, the two heads issued
    back-to-back to complementary row groups (tile_position (0,0)/(64,0)) so
    they run concurrently in the PE array -- 2x QK throughput vs zero-padding
    the contraction to 128.
  - exp on ScalarE in groups of up to 3 sk-tiles (one 3-bank PSUM tile per
    group) to amortize the ~293ns fixed ACTIVATE overhead.
  - PV accumulates oT[65, sq] per head (row 64 = softmax denominator via the
    mask column).  NO on-device transpose or normalization: the kernel ships
    oT + denominator to DRAM and the HOST does o = (num/den).T and the
    Q_len row masking during unsharding.  This removes the fp32 PE-transpose
    matmuls (~80us of PE time in the old kernel) entirely.
  - Emission is software-pipelined: the next batch's projection pieces and
    the previous chunk's PV pieces are interleaved between QK groups to keep
    the in-order PE queue dense while ScalarE (the attention-phase
    bottleneck) drains the exp queue.

The program is compiled for the runtime tile counts (SQT_b, SKT_b) =
ceil(len/128) per batch (shared SPMD program across the 8 cores).
"""

import math

import numpy as np
import ml_dtypes


def _ensure_paths():
    import sys
    try:
        import concourse  # noqa: F401
        return
    except ImportError:
        pass
    for p in ("/opt/trn_rl_repo", "/root/.axon_site/_ro/trn_rl_repo"):
        if p not in sys.path:
            sys.path.insert(0, p)
    import concourse  # noqa: F401


P = 128          # SBUF partitions
D = 1024         # model dim
DH = 64          # head dim
KT = D // P      # contraction tiles for projections
GN = 3           # sk-tiles per exp group (3 PSUM banks)
NB = 4           # batches
NCORES = 8

_PROG_CACHE = {}

# exposed for test.py profiling reruns
_last_nc = None
_last_in_maps = None


def _chunks(total, sz=512):
    out = []
    o = 0
    while o < total:
        n = min(sz, total - o)
        out.append((o, n))
        o += n
    return out


def _build_program(SQT, SKT):
    """Build + bacc-compile the shared SPMD program for given per-batch tile
    counts (SQT, SKT are 4-tuples)."""
    _ensure_paths()
    import concourse.bass as bass  # noqa: F401
    import concourse.tile as tile
    from concourse import bacc, mybir

    BF = mybir.dt.bfloat16
    F32 = mybir.dt.float32
    Exp = mybir.ActivationFunctionType.Exp

    SQ = [t * P for t in SQT]
    SK = [t * P for t in SKT]
    SQtot = sum(SQ)
    SKtot = sum(SK)
    QOFF = [sum(SQ[:b]) for b in range(NB)]
    KOFF = [sum(SK[:b]) for b in range(NB)]
    TOFF = [sum(SKT[:b]) for b in range(NB)]
    SKTtot = sum(SKT)
    ATM = max(SKT)

    nc = bacc.Bacc("TRN2", target_bir_lowering=False, debug=False,
                   num_devices=NCORES)

    xq = nc.dram_tensor("xq", [D, SQtot], BF, kind="ExternalInput").ap()
    xk = nc.dram_tensor("xk", [D, SKtot], BF, kind="ExternalInput").ap()
    xv = nc.dram_tensor("xv", [D, SKtot], BF, kind="ExternalInput").ap()
    wq = nc.dram_tensor("wq", [D, P], BF, kind="ExternalInput").ap()
    wk = nc.dram_tensor("wk", [D, P], BF, kind="ExternalInput").ap()
    wv = nc.dram_tensor("wv", [D, P], BF, kind="ExternalInput").ap()
    mk2 = nc.dram_tensor("mk2", [P, SKTtot, 2], BF, kind="ExternalInput").ap()
    out = nc.dram_tensor("out", [2, DH + 1, SQtot], F32,
                         kind="ExternalOutput").ap()

    xq_r = xq.rearrange("(k p) s -> p k s", p=P)
    xk_r = xk.rearrange("(k p) s -> p k s", p=P)
    xv_r = xv.rearrange("(k p) s -> p k s", p=P)

    VW = 2 * (DH + 1)        # 130: [A num 64 | A mask | B num 64 | B mask]

    with tile.TileContext(nc) as tc:
        with tc.tile_pool(name="const", bufs=1) as const, \
             tc.tile_pool(name="persist", bufs=1) as persist, \
             tc.tile_pool(name="xs", bufs=10) as xs, \
             tc.tile_pool(name="atp", bufs=3) as atp, \
             tc.tile_pool(name="otp", bufs=2) as otp, \
             tc.tile_pool(name="psq", bufs=2, space="PSUM") as psq, \
             tc.tile_pool(name="pss", bufs=2, space="PSUM") as pss:

            wq_sb = const.tile([P, KT, P], BF, tag="wq")
            wk_sb = const.tile([P, KT, P], BF, tag="wk")
            wv_sb = const.tile([P, KT, P], BF, tag="wv")
            mask_sb = const.tile([P, SKTtot, 2], BF, tag="mk")
            # warm-up tile memset FIRST on the gpsimd engine (it is idle and
            # ready before Vector; and it must precede the w DMAs below so
            # the PE warm-up isn't queued behind their descriptor time)
            warm = const.tile([P, 512], BF, tag="warm")
            nc.gpsimd.memset(warm, 0.0)
            # wk on the sync queue ahead of the x stream (first QK group
            # needs it); wq/wv/mask via gpsimd so xk0 starts 0.8us earlier
            nc.sync.dma_start(out=wk_sb, in_=wk.rearrange("(k p) e -> p k e", p=P))
            nc.gpsimd.dma_start(out=wq_sb, in_=wq.rearrange("(k p) e -> p k e", p=P))
            nc.gpsimd.dma_start(out=wv_sb, in_=wv.rearrange("(k p) e -> p k e", p=P))
            nc.gpsimd.dma_start(out=mask_sb, in_=mk2)

            qt = persist.tile([P, SQtot], BF, tag="qt")
            kt = persist.tile([P, SKtot], BF, tag="kt")
            v_sb = persist.tile([P, SKTtot, VW], BF, tag="v")

            # PE warm-up: dummy matmuls with no input dependencies.
            # They run during the initial input-DMA wait and flip the HAM
            # clock gate to 8/8 so the first real matmuls run at 2.4 GHz.
            for w in range(20):
                wp = pss.tile([P, 512], F32, tag="acc", name=f"warm_{w}")
                nc.tensor.matmul(wp, warm[:, 0:P], warm,
                                 start=True, stop=True)

            # process batches LARGEST-first: the big batch's long exp phase
            # covers the later batches' input DMA, and only kT + q0 of the
            # first batch gate the start of attention
            BORD = sorted(range(NB), key=lambda b: -SQT[b] * SKT[b])

            # attention / q-proj chunk lists: a small first chunk for the
            # first batch (less DMA before the first exp) and a small final
            # chunk for the last batch (shorter PV tail)
            chl = {b: _chunks(SQ[b]) for b in range(NB)}
            kchl = {b: _chunks(SK[b]) for b in range(NB)}
            if SQ[BORD[0]] > 512:
                chl[BORD[0]] = [(0, 256)] + \
                    [(256 + o, n) for o, n in _chunks(SQ[BORD[0]] - 256)]
            if SK[BORD[0]] > 512:
                kchl[BORD[0]] = [(0, 384)] + \
                    [(384 + o, n) for o, n in _chunks(SK[BORD[0]] - 384)]
            _lo, _ln = chl[BORD[-1]][-1]
            if _ln > 128:
                chl[BORD[-1]] = chl[BORD[-1]][:-1] + \
                    [(_lo, _ln - 128), (_lo + _ln - 128, 128)]

            # ---------------- input-DMA pump + mm pieces ----------------
            # The x-input DMAs are DECOUPLED from the matmul pieces: all
            # transfers are emitted in need-order on the ONE sync queue,
            # ~XS_DEPTH pieces ahead of their consumers (the xs ring
            # throttles at runtime).  The queue then never idles, which
            # matters because the input stream (~26 MB at ~330 GB/s) has
            # almost no slack against the compute schedule.
            XS_DEPTH = 10
            dma_stream = []      # need-ordered [(key, emit_fn)]
            dma_emitted = {}     # key -> xt tile (set when DMA emitted)
            mm_done_idx = set()  # stream indices whose mm part has run
            pump_state = [0]     # next stream index to emit

            def _mk_dma(key, src_r, off, c0, n):
                def emit():
                    if key in dma_emitted:
                        return
                    xt = xs.tile([P, KT, 512], BF, tag="x")
                    nc.sync.dma_start(out=xt[:, :, :n],
                                      in_=src_r[:, :, off + c0:off + c0 + n])
                    dma_emitted[key] = xt
                return (key, emit)

            def pump_dma():
                """Emit pending input DMAs: stream index i may be emitted
                once the reader of ring slot i-XS_DEPTH has been emitted
                (pool WAR dep must point backward in program order)."""
                while pump_state[0] < len(dma_stream):
                    i = pump_state[0]
                    if i >= XS_DEPTH and i - XS_DEPTH not in mm_done_idx:
                        return
                    dma_stream[i][1]()
                    pump_state[0] = i + 1

            stream_idx = {}      # key -> position in dma_stream

            def q_proj_piece(b, c0, n, dst, w_sb, off, kind):
                key = (kind, b, c0)

                def go():
                    if key not in dma_emitted:       # safety: force-emit
                        dma_stream[stream_idx[key]][1]()
                    xt = dma_emitted[key]
                    ps = pss.tile([P, 512], F32, tag="acc")
                    for k in range(KT):
                        nc.tensor.matmul(ps[:, :n], w_sb[:, k, :], xt[:, k, :n],
                                         start=(k == 0), stop=(k == KT - 1))
                    nc.vector.tensor_copy(out=dst[:, off + c0:off + c0 + n],
                                          in_=ps[:, :n])
                    mm_done_idx.add(stream_idx[key])
                    pump_dma()
                return (key, go)

            def v_proj_piece(b, c0, n):
                key = ('v', b, c0)

                def go():
                    if key not in dma_emitted:       # safety: force-emit
                        dma_stream[stream_idx[key]][1]()
                    xt = dma_emitted[key]
                    nt = n // P
                    ta = TOFF[b] + c0 // P
                    for st in range(nt):
                        ps = pss.tile([P, 512], F32, tag="acc")
                        for k in range(KT):
                            nc.tensor.matmul(
                                ps[:, :P], xt[:, k, P * st:P * (st + 1)],
                                wv_sb[:, k, :],
                                start=(k == 0), stop=(k == KT - 1))
                        vt = v_sb[:, ta + st, 0:VW].rearrange(
                            "p (g c) -> p g c", c=DH + 1)
                        nc.vector.tensor_copy(
                            out=vt[:, :, 0:DH],
                            in_=ps[:, :P].rearrange("p (g c) -> p g c", c=DH))
                        nc.vector.tensor_copy(
                            out=vt[:, :, DH:DH + 1],
                            in_=mask_sb[:, ta + st:ta + st + 1, :].rearrange(
                                "p t g -> p g t"))
                    mm_done_idx.add(stream_idx[key])
                    pump_dma()
                return (key, go)

            # per-batch piece lists, created ONCE (each emits exactly once)
            chunk_n = {}
            QP, KP, VP = {}, {}, {}
            for b in range(NB):
                QP[b] = [q_proj_piece(b, c0, n, qt, wq_sb, QOFF[b], 'q')
                         for c0, n in chl[b]]
                KP[b] = [q_proj_piece(b, c0, n, kt, wk_sb, KOFF[b], 'k')
                         for c0, n in kchl[b]]
                VP[b] = [v_proj_piece(b, c0, n) for c0, n in _chunks(SK[b])]
                for c0, n in chl[b]:
                    chunk_n[('q', b, c0)] = n
                for c0, n in kchl[b]:
                    chunk_n[('k', b, c0)] = n
                for c0, n in _chunks(SK[b]):
                    chunk_n[('v', b, c0)] = n

            def prelude_a(b):
                """What batch b's first QK group needs: first kT chunk + the
                first q chunk."""
                return [KP[b][0], QP[b][0]]

            def prelude_rest(b):
                return KP[b][1:]

            def prelude_b(b):
                """v projection -- only needed by PV, one chunk after QK."""
                return VP[b]

            # ---------------- attention ----------------
            def pv_pieces(b, c0, n, ats, tail_dma=False):
                """PV + evac + output-DMA closures for one finished chunk."""
                pieces = []

                def mk_pv(g, t0, t1, po_box):
                    def go():
                        if t0 == 0:
                            po_box[0] = pss.tile([P, 512], F32, tag="acc",
                                                 name=f"po_{b}_{c0}_{g}")
                        po = po_box[0]
                        for t in range(t0, t1):
                            # lhsT is only the 65 live columns: LDWEIGHTS
                            # time scales with column count
                            nc.tensor.matmul(
                                po[0:DH + 1, :n],
                                v_sb[:, TOFF[b] + t, (DH + 1) * g:
                                     (DH + 1) * (g + 1)],
                                ats[g][:, t, :n],
                                start=(t == 0), stop=(t == SKT[b] - 1))
                        if t1 == SKT[b]:
                            ot = otp.tile([DH + 1, 512], F32, tag="ot",
                                          name=f"ot_{b}_{c0}_{g}")
                            nc.vector.tensor_copy(out=ot[:, :n],
                                                  in_=po[0:DH + 1, :n])
                            # output DMA on the (idle) GpSimd queue so it
                            # never blocks input prefetch on the sync queue;
                            # the tail chunks go via the scalar queue so the
                            # final flush drains two queues in parallel
                            eng = nc.scalar if tail_dma else nc.gpsimd
                            eng.dma_start(
                                out=out[g, :, QOFF[b] + c0:QOFF[b] + c0 + n],
                                in_=ot[:, :n])
                    return go

                for g in range(2):
                    box = [None]
                    if SKT[b] > 1:
                        half = (SKT[b] + 1) // 2
                        pieces.append((None, mk_pv(g, 0, half, box)))
                        pieces.append((None, mk_pv(g, half, SKT[b], box)))
                    else:
                        pieces.append((None, mk_pv(g, 0, SKT[b], box)))
                return pieces

            # PV pieces of chunk c are emitted two chunks later (the at
            # ring is 3 deep), so side-work DMA has two chunks of exp time
            # to stream in before it can stall the PE queue
            pendq = []         # FIFO of per-chunk PV piece lists

            def emit_chunk(b, c0, n, fresh, tail=False):
                """Emit one attention chunk; `fresh` (proj pieces) and the
                due PV pieces are interleaved between QK groups."""
                ats = (atp.tile([P, ATM, 512], BF, tag="ata",
                                name=f"ata_{b}_{c0}"),
                       atp.tile([P, ATM, 512], BF, tag="atb",
                                name=f"atb_{b}_{c0}"))
                groups = [(t0, min(GN, SKT[b] - t0))
                          for t0 in range(0, SKT[b], GN)]
                side = list(fresh)
                if len(pendq) >= 1:
                    side = side + pendq.pop(0)
                L = len(side)
                done = 0
                for gi, (t0, gn) in enumerate(groups):
                    pq = [psq.tile([P, GN, 512], F32, tag="qk",
                                   name=f"qk_{b}_{c0}_{t0}_{g}")
                          for g in range(2)]
                    for j in range(gn):
                        t = t0 + j
                        for g in range(2):
                            nc.tensor.matmul(
                                pq[g][:, j, :n],
                                kt[DH * g:DH * (g + 1),
                                   KOFF[b] + P * t:KOFF[b] + P * (t + 1)],
                                qt[DH * g:DH * (g + 1),
                                   QOFF[b] + c0:QOFF[b] + c0 + n],
                                start=True, stop=True)
                    for g in range(2):
                        nc.scalar.activation(
                            out=ats[g][:, t0:t0 + gn, :n],
                            in_=pq[g][:, 0:gn, :n],
                            func=Exp, scale=0.125)
                    upto = (L * (gi + 1)) // len(groups)
                    while done < upto:
                        side[done][1]()
                        done += 1
                while done < L:
                    side[done][1]()
                    done += 1
                pendq.append(pv_pieces(b, c0, n, ats, tail))

            # ---- pass 1: build the chunk schedule (no emission) ----
            # carry_kt = batch's remaining kT chunks: chunk 0's later QK
            # groups need them, so they lead chunk 0's side.  carry_v = its
            # v pieces: must emit before PV of chunk 0 -> spread over
            # chunks 0 and 1.  q-proj pieces run two chunks ahead of their
            # QK consumer.  filler = next batch's k/q0 prefetch.
            sched = []
            carry_kt = prelude_rest(BORD[0])
            carry_v = prelude_b(BORD[0])
            for bi, b in enumerate(BORD):
                nxt = BORD[bi + 1] if bi + 1 < NB else None
                kt0_, v0_ = carry_kt, carry_v
                filler = (prelude_a(nxt) + prelude_rest(nxt)) \
                    if nxt is not None else []
                carry_kt = []
                carry_v = prelude_b(nxt) if nxt is not None else []
                qr = QP[b][1:]
                ch = chl[b]
                vh = (len(v0_) + 1) // 2 if len(ch) > 1 else len(v0_)
                fdone = 0
                for ci, (c0, n) in enumerate(ch):
                    take = (len(filler) * (ci + 1)) // len(ch) - fdone
                    if ci == 0:
                        qf = qr[0:2]
                    else:
                        qf = qr[ci + 1:ci + 2]
                    fresh = (kt0_ + v0_[:vh] if ci == 0 else []) \
                        + (v0_[vh:] if ci == 1 else []) \
                        + qf \
                        + filler[fdone:fdone + take]
                    fdone += take
                    sched.append((b, c0, n, fresh,
                                  bi == NB - 1 and ci >= len(ch) - 2))

            # ---- input stream in EXACT mm-consumption order ----
            srcs = {'k': xk_r, 'q': xq_r, 'v': xv_r}
            offs = {'k': KOFF, 'q': QOFF, 'v': KOFF}
            for key, _go in prelude_a(BORD[0]):
                stream_idx[key] = len(dma_stream)
                dma_stream.append(_mk_dma(
                    key, srcs[key[0]], offs[key[0]][key[1]], key[2],
                    chunk_n[key]))
            for _b, _c0, _n, fresh, _tl in sched:
                for key, _go in fresh:
                    if key is None or key in stream_idx:
                        continue
                    stream_idx[key] = len(dma_stream)
                    dma_stream.append(_mk_dma(
                        key, srcs[key[0]], offs[key[0]][key[1]], key[2],
                        chunk_n[key]))
            pump_dma()           # emit the initial XS_DEPTH transfers

            # ---- pass 2: emission ----
            for _key, go in prelude_a(BORD[0]):
                go()
            for b, c0, n, fresh, tail in sched:
                emit_chunk(b, c0, n, fresh, tail)
            for lst in pendq:
                for _key, go in lst:
                    go()
            pendq.clear()

    nc.compile()
    return nc


def _get_program(SQT, SKT):
    key = (tuple(SQT), tuple(SKT))
    if key not in _PROG_CACHE:
        _PROG_CACHE[key] = _build_program(key[0], key[1])
    return _PROG_CACHE[key]


def _prep_inputs(Q_seq, K_seq, V_seq, WQ, WK, WV, Q_len, V_len):
    """Host-side shared prep: per-batch transposed bf16 activations and
    masks, concatenated along seq; returns (SQT, SKT, shared dict)."""
    BF = ml_dtypes.bfloat16
    B = Q_seq.shape[0]
    SQT = [max(1, math.ceil(int(Q_len[b]) / P)) for b in range(B)]
    SKT = [max(1, math.ceil(int(V_len[b]) / P)) for b in range(B)]
    SQ = [t * P for t in SQT]
    SK = [t * P for t in SKT]

    xq = np.concatenate(
        [np.ascontiguousarray(Q_seq[b, :SQ[b]].T) for b in range(B)],
        axis=1).astype(BF)
    xk = np.concatenate(
        [np.ascontiguousarray(K_seq[b, :SK[b]].T) for b in range(B)],
        axis=1).astype(BF)
    mks = [(np.arange(SK[b]) < int(V_len[b])) for b in range(B)]
    xv = np.concatenate(
        [np.ascontiguousarray((V_seq[b, :SK[b]] * mks[b][:, None]).T)
         for b in range(B)], axis=1).astype(BF)
    # mask laid out partition-major [128, SKTtot, 2] so the DMA moves
    # contiguous per-partition lines
    mkcat = np.concatenate(mks)                       # [SKtot]
    mk2 = np.repeat(
        mkcat.reshape(-1, P).T[:, :, None], 2, axis=2).astype(BF)
    return SQT, SKT, {"xq": xq, "xk": xk, "xv": xv, "mk2": mk2}


def kernel(Q_seq, K_seq, V_seq, WQ, WK, WV, Q_len, V_len):
    global _last_nc, _last_in_maps
    _ensure_paths()
    from concourse.bass_utils import run_bass_kernel_spmd

    Q_seq = np.asarray(Q_seq, dtype=np.float32)
    K_seq = np.asarray(K_seq, dtype=np.float32)
    V_seq = np.asarray(V_seq, dtype=np.float32)
    WQ = np.asarray(WQ, dtype=np.float32)
    WK = np.asarray(WK, dtype=np.float32)
    WV = np.asarray(WV, dtype=np.float32)
    Q_len = np.asarray(Q_len).reshape(-1)
    V_len = np.asarray(V_len).reshape(-1)

    B, S, _ = Q_seq.shape
    BF = ml_dtypes.bfloat16

    SQT, SKT, shared = _prep_inputs(Q_seq, K_seq, V_seq, WQ, WK, WV,
                                    Q_len, V_len)
    SQ = [t * P for t in SQT]
    QOFF = [sum(SQ[:b]) for b in range(B)]

    nc = _get_program(SQT, SKT)

    in_maps = []
    for c in range(NCORES):
        m = dict(shared)
        m["wq"] = np.ascontiguousarray(WQ[:, P * c:P * (c + 1)]).astype(BF)
        m["wk"] = np.ascontiguousarray(WK[:, P * c:P * (c + 1)]).astype(BF)
        m["wv"] = np.ascontiguousarray(WV[:, P * c:P * (c + 1)]).astype(BF)
        in_maps.append(m)

    res = run_bass_kernel_spmd(nc, in_maps, core_ids=list(range(NCORES)))
    _last_nc, _last_in_maps = nc, in_maps

    H = 16
    full = np.zeros((B, S, H * DH), dtype=np.float32)
    for c in range(NCORES):
        o = res.results[c]["out"]          # [2, 65, SQtot]
        for g in range(2):
            h = 2 * c + g
            num = o[g, :DH]                # [64, SQtot]
            den = o[g, DH:DH + 1]          # [1, SQtot]
            ot = num / den
            for b in range(B):
                ql = int(Q_len[b])
                sl = ot[:, QOFF[b]:QOFF[b] + SQ[b]]
                full[b, :SQ[b], DH * h:DH * (h + 1)] = sl.T
                full[b, ql:, DH * h:DH * (h + 1)] = 0.0
    return full



# revision 33
# speedup vs baseline: 1.2229x; 1.0036x over previous
"""Multi-head attention (B=4, S=2048, D=1024, H=16, Dh=64) on 8 trn2 NeuronCores.

Sharding: core c -> heads (2c, 2c+1) of ALL 4 batches.  Every batch has 16
heads, so each core gets exactly 2 heads x 4 batches and per-core attention
work is Sum_b SQT_b*SKT_b score tiles -- perfectly balanced across cores
regardless of the per-batch sequence lengths (the old batch-sharded layout
made the largest-batch core ~2.1x slower than the mean).

Per core (2 heads, head A on partitions 0:64, head B on 64:128):
  - Host pre-transposes X per batch (D-major) in bf16 and concatenates the
    batches along seq: xq [D, SQtot], xk/xv [D, SKtot] (V rows >= V_len are
    zeroed on host).
  - Projections: qT/kT in [dh, seq] orientation, v in natural [seq, dh]
    orientation with a mask column appended per head (denominator trick).
  - QK computes scoresT[sk, sq] with K=64 contraction, the two heads issued
    back-to-back to complementary row groups (tile_position (0,0)/(64,0)) so
    they run concurrently in the PE array -- 2x QK throughput vs zero-padding
    the contraction to 128.
  - exp on ScalarE in groups of up to 3 sk-tiles (one 3-bank PSUM tile per
    group) to amortize the ~293ns fixed ACTIVATE overhead.
  - PV accumulates oT[65, sq] per head (row 64 = softmax denominator via the
    mask column).  NO on-device transpose or normalization: the kernel ships
    oT + denominator to DRAM and the HOST does o = (num/den).T and the
    Q_len row masking during unsharding.  This removes the fp32 PE-transpose
    matmuls (~80us of PE time in the old kernel) entirely.
  - Emission is software-pipelined: the next batch's projection pieces and
    the previous chunk's PV pieces are interleaved between QK groups to keep
    the in-order PE queue dense while ScalarE (the attention-phase
    bottleneck) drains the exp queue.

The program is compiled for the runtime tile counts (SQT_b, SKT_b) =
ceil(len/128) per batch (shared SPMD program across the 8 cores).
"""

import math

import numpy as np
import ml_dtypes


def _ensure_paths():
    import sys
    try:
        import concourse  # noqa: F401
        return
    except ImportError:
        pass
    for p in ("/opt/trn_rl_repo", "/root/.axon_site/_ro/trn_rl_repo"):
        if p not in sys.path:
            sys.path.insert(0, p)
    import concourse  # noqa: F401


P = 128          # SBUF partitions
D = 1024         # model dim
DH = 64          # head dim
KT = D // P      # contraction tiles for projections
GN = 3           # sk-tiles per exp group (3 PSUM banks)
NB = 4           # batches
NCORES = 8

_PROG_CACHE = {}

# exposed for test.py profiling reruns
_last_nc = None
_last_in_maps = None


def _chunks(total, sz=512):
    out = []
    o = 0
    while o < total:
        n = min(sz, total - o)
        out.append((o, n))
        o += n
    return out


def _build_program(SQT, SKT):
    """Build + bacc-compile the shared SPMD program for given per-batch tile
    counts (SQT, SKT are 4-tuples)."""
    _ensure_paths()
    import concourse.bass as bass  # noqa: F401
    import concourse.tile as tile
    from concourse import bacc, mybir

    BF = mybir.dt.bfloat16
    F32 = mybir.dt.float32
    Exp = mybir.ActivationFunctionType.Exp

    SQ = [t * P for t in SQT]
    SK = [t * P for t in SKT]
    SQtot = sum(SQ)
    SKtot = sum(SK)
    QOFF = [sum(SQ[:b]) for b in range(NB)]
    KOFF = [sum(SK[:b]) for b in range(NB)]
    TOFF = [sum(SKT[:b]) for b in range(NB)]
    SKTtot = sum(SKT)
    ATM = max(SKT)

    nc = bacc.Bacc("TRN2", target_bir_lowering=False, debug=False,
                   num_devices=NCORES)

    xq = nc.dram_tensor("xq", [D, SQtot], BF, kind="ExternalInput").ap()
    xk = nc.dram_tensor("xk", [D, SKtot], BF, kind="ExternalInput").ap()
    xv = nc.dram_tensor("xv", [D, SKtot], BF, kind="ExternalInput").ap()
    wq = nc.dram_tensor("wq", [D, P], BF, kind="ExternalInput").ap()
    wk = nc.dram_tensor("wk", [D, P], BF, kind="ExternalInput").ap()
    wv = nc.dram_tensor("wv", [D, P], BF, kind="ExternalInput").ap()
    mk2 = nc.dram_tensor("mk2", [P, SKTtot, 2], BF, kind="ExternalInput").ap()
    # bf16 output: halves the output DMA and the tail flush; the host
    # division num/den happens in fp32 so only ~0.3% quantization noise
    # is added on top of the bf16 matmul pipeline (gate is 2e-2)
    out = nc.dram_tensor("out", [2, DH + 1, SQtot], BF,
                         kind="ExternalOutput").ap()

    xq_r = xq.rearrange("(k p) s -> p k s", p=P)
    xk_r = xk.rearrange("(k p) s -> p k s", p=P)
    xv_r = xv.rearrange("(k p) s -> p k s", p=P)

    VW = 2 * (DH + 1)        # 130: [A num 64 | A mask | B num 64 | B mask]

    with tile.TileContext(nc) as tc:
        with tc.tile_pool(name="const", bufs=1) as const, \
             tc.tile_pool(name="persist", bufs=1) as persist, \
             tc.tile_pool(name="xs", bufs=10) as xs, \
             tc.tile_pool(name="atp", bufs=3) as atp, \
             tc.tile_pool(name="otp", bufs=2) as otp, \
             tc.tile_pool(name="psq", bufs=2, space="PSUM") as psq, \
             tc.tile_pool(name="pss", bufs=2, space="PSUM") as pss:

            wq_sb = const.tile([P, KT, P], BF, tag="wq")
            wk_sb = const.tile([P, KT, P], BF, tag="wk")
            wv_sb = const.tile([P, KT, P], BF, tag="wv")
            mask_sb = const.tile([P, SKTtot, 2], BF, tag="mk")
            # warm-up tile memset FIRST on the gpsimd engine (it is idle and
            # ready before Vector; and it must precede the w DMAs below so
            # the PE warm-up isn't queued behind their descriptor time)
            warm = const.tile([P, 512], BF, tag="warm")
            nc.gpsimd.memset(warm, 0.0)
            # wk on the sync queue ahead of the x stream (first QK group
            # needs it); wq/wv/mask via gpsimd so xk0 starts 0.8us earlier
            nc.sync.dma_start(out=wk_sb, in_=wk.rearrange("(k p) e -> p k e", p=P))
            nc.gpsimd.dma_start(out=wq_sb, in_=wq.rearrange("(k p) e -> p k e", p=P))
            nc.gpsimd.dma_start(out=wv_sb, in_=wv.rearrange("(k p) e -> p k e", p=P))
            nc.gpsimd.dma_start(out=mask_sb, in_=mk2)

            qt = persist.tile([P, SQtot], BF, tag="qt")
            kt = persist.tile([P, SKtot], BF, tag="kt")
            v_sb = persist.tile([P, SKTtot, VW], BF, tag="v")
            # the mask (denominator) columns of v_sb are constant: write
            # them all with ONE strided copy instead of 2 tiny DVE copies
            # per sk-subtile in the hot path
            nc.vector.tensor_copy(
                out=v_sb.rearrange("p t (g c) -> p t g c", c=DH + 1)[:, :, :, DH],
                in_=mask_sb)

            # PE warm-up: dummy matmuls with no input dependencies.
            # They run during the initial input-DMA wait and flip the HAM
            # clock gate to 8/8 so the first real matmuls run at 2.4 GHz.
            # All 20 target ONE psq-pool tile: rotating the pss pool here
            # would make the first projections wait on warm-up completions.
            wp = psq.tile([P, GN, 512], F32, tag="qk", name="warm_ps")
            for w in range(20):
                nc.tensor.matmul(wp[:, 0, :], warm[:, 0:P], warm,
                                 start=True, stop=True)

            # process batches LARGEST-first: the big batch's long exp phase
            # covers the later batches' input DMA, and only kT + q0 of the
            # first batch gate the start of attention
            BORD = sorted(range(NB), key=lambda b: -SQT[b] * SKT[b])

            # attention / q-proj chunk lists: a small first chunk for the
            # first batch (less DMA before the first exp) and a small final
            # chunk for the last batch (shorter PV tail)
            chl = {b: _chunks(SQ[b]) for b in range(NB)}
            kchl = {b: _chunks(SK[b]) for b in range(NB)}
            if SQ[BORD[0]] > 512:
                chl[BORD[0]] = [(0, 256)] + \
                    [(256 + o, n) for o, n in _chunks(SQ[BORD[0]] - 256)]
            if SK[BORD[0]] > 512:
                kchl[BORD[0]] = [(0, 384)] + \
                    [(384 + o, n) for o, n in _chunks(SK[BORD[0]] - 384)]
            _lo, _ln = chl[BORD[-1]][-1]
            if _ln > 128:
                chl[BORD[-1]] = chl[BORD[-1]][:-1] + \
                    [(_lo, _ln - 128), (_lo + _ln - 128, 128)]

            # ---------------- input-DMA pump + mm pieces ----------------
            # The x-input DMAs are DECOUPLED from the matmul pieces: all
            # transfers are emitted in need-order on the ONE sync queue,
            # ~XS_DEPTH pieces ahead of their consumers (the xs ring
            # throttles at runtime).  The queue then never idles, which
            # matters because the input stream (~26 MB at ~330 GB/s) has
            # almost no slack against the compute schedule.
            XS_DEPTH = 10
            dma_stream = []      # need-ordered [(key, emit_fn)]
            dma_emitted = {}     # key -> xt tile (set when DMA emitted)
            mm_done_idx = set()  # stream indices whose mm part has run
            pump_state = [0]     # next stream index to emit

            def _mk_dma(key, src_r, off, c0, n):
                def emit():
                    if key in dma_emitted:
                        return
                    xt = xs.tile([P, KT, 512], BF, tag="x")
                    nc.sync.dma_start(out=xt[:, :, :n],
                                      in_=src_r[:, :, off + c0:off + c0 + n])
                    dma_emitted[key] = xt
                return (key, emit)

            def pump_dma():
                """Emit pending input DMAs: stream index i may be emitted
                once the reader of ring slot i-XS_DEPTH has been emitted
                (pool WAR dep must point backward in program order)."""
                while pump_state[0] < len(dma_stream):
                    i = pump_state[0]
                    if i >= XS_DEPTH and i - XS_DEPTH not in mm_done_idx:
                        return
                    dma_stream[i][1]()
                    pump_state[0] = i + 1

            stream_idx = {}      # key -> position in dma_stream

            def q_proj_piece(b, c0, n, dst, w_sb, off, kind):
                key = (kind, b, c0)

                def go():
                    if key not in dma_emitted:       # safety: force-emit
                        dma_stream[stream_idx[key]][1]()
                    xt = dma_emitted[key]
                    ps = pss.tile([P, 512], F32, tag="acc")
                    for k in range(KT):
                        nc.tensor.matmul(ps[:, :n], w_sb[:, k, :], xt[:, k, :n],
                                         start=(k == 0), stop=(k == KT - 1))
                    nc.vector.tensor_copy(out=dst[:, off + c0:off + c0 + n],
                                          in_=ps[:, :n])
                    mm_done_idx.add(stream_idx[key])
                    pump_dma()
                return (key, go)

            def v_proj_piece(b, c0, n):
                key = ('v', b, c0)

                def go():
                    if key not in dma_emitted:       # safety: force-emit
                        dma_stream[stream_idx[key]][1]()
                    xt = dma_emitted[key]
                    nt = n // P
                    ta = TOFF[b] + c0 // P
                    for st in range(nt):
                        ps = pss.tile([P, 512], F32, tag="acc")
                        for k in range(KT):
                            nc.tensor.matmul(
                                ps[:, :P], xt[:, k, P * st:P * (st + 1)],
                                wv_sb[:, k, :],
                                start=(k == 0), stop=(k == KT - 1))
                        vt = v_sb[:, ta + st, 0:VW].rearrange(
                            "p (g c) -> p g c", c=DH + 1)
                        nc.vector.tensor_copy(
                            out=vt[:, :, 0:DH],
                            in_=ps[:, :P].rearrange("p (g c) -> p g c", c=DH))
                    mm_done_idx.add(stream_idx[key])
                    pump_dma()
                return (key, go)

            # per-batch piece lists, created ONCE (each emits exactly once)
            chunk_n = {}
            QP, KP, VP = {}, {}, {}
            for b in range(NB):
                QP[b] = [q_proj_piece(b, c0, n, qt, wq_sb, QOFF[b], 'q')
                         for c0, n in chl[b]]
                KP[b] = [q_proj_piece(b, c0, n, kt, wk_sb, KOFF[b], 'k')
                         for c0, n in kchl[b]]
                VP[b] = [v_proj_piece(b, c0, n) for c0, n in _chunks(SK[b])]
                for c0, n in chl[b]:
                    chunk_n[('q', b, c0)] = n
                for c0, n in kchl[b]:
                    chunk_n[('k', b, c0)] = n
                for c0, n in _chunks(SK[b]):
                    chunk_n[('v', b, c0)] = n

            def prelude_a(b):
                """What batch b's first QK group needs: first kT chunk + the
                first q chunk."""
                return [KP[b][0], QP[b][0]]

            def prelude_rest(b):
                return KP[b][1:]

            def prelude_b(b):
                """v projection -- only needed by PV, one chunk after QK."""
                return VP[b]

            # ---------------- attention ----------------
            def pv_pieces(b, c0, n, ats, tail_dma=False):
                """PV + evac + output-DMA closures for one finished chunk."""
                pieces = []

                def mk_pv(g, t0, t1, po_box):
                    def go():
                        if t0 == 0:
                            po_box[0] = pss.tile([P, 512], F32, tag="acc",
                                                 name=f"po_{b}_{c0}_{g}")
                        po = po_box[0]
                        for t in range(t0, t1):
                            # lhsT is only the 65 live columns: LDWEIGHTS
                            # time scales with column count
                            nc.tensor.matmul(
                                po[0:DH + 1, :n],
                                v_sb[:, TOFF[b] + t, (DH + 1) * g:
                                     (DH + 1) * (g + 1)],
                                ats[g][:, t, :n],
                                start=(t == 0), stop=(t == SKT[b] - 1))
                        if t1 == SKT[b]:
                            ot = otp.tile([DH + 1, 512], BF, tag="ot",
                                          name=f"ot_{b}_{c0}_{g}")
                            nc.vector.tensor_copy(out=ot[:, :n],
                                                  in_=po[0:DH + 1, :n])
                            # output DMA on the (idle) GpSimd queue so it
                            # never blocks input prefetch on the sync queue;
                            # the tail chunks go via the scalar queue so the
                            # final flush drains two queues in parallel
                            eng = nc.scalar if tail_dma else nc.gpsimd
                            eng.dma_start(
                                out=out[g, :, QOFF[b] + c0:QOFF[b] + c0 + n],
                                in_=ot[:, :n])
                    return go

                for g in range(2):
                    box = [None]
                    if SKT[b] > 1:
                        half = (SKT[b] + 1) // 2
                        pieces.append((None, mk_pv(g, 0, half, box)))
                        pieces.append((None, mk_pv(g, half, SKT[b], box)))
                    else:
                        pieces.append((None, mk_pv(g, 0, SKT[b], box)))
                return pieces

            # PV pieces of chunk c are emitted two chunks later (the at
            # ring is 3 deep), so side-work DMA has two chunks of exp time
            # to stream in before it can stall the PE queue
            pendq = []         # FIFO of per-chunk PV piece lists

            def emit_chunk(b, c0, n, fresh, tail=False):
                """Emit one attention chunk; `fresh` (proj pieces) and the
                due PV pieces are interleaved between QK groups."""
                ats = (atp.tile([P, ATM, 512], BF, tag="ata",
                                name=f"ata_{b}_{c0}"),
                       atp.tile([P, ATM, 512], BF, tag="atb",
                                name=f"atb_{b}_{c0}"))
                groups = [(t0, min(GN, SKT[b] - t0))
                          for t0 in range(0, SKT[b], GN)]
                side = list(fresh)
                if len(pendq) >= 1:
                    side = side + pendq.pop(0)
                L = len(side)
                done = 0
                for gi, (t0, gn) in enumerate(groups):
                    pq = [psq.tile([P, GN, 512], F32, tag="qk",
                                   name=f"qk_{b}_{c0}_{t0}_{g}")
                          for g in range(2)]
                    for j in range(gn):
                        t = t0 + j
                        for g in range(2):
                            nc.tensor.matmul(
                                pq[g][:, j, :n],
                                kt[DH * g:DH * (g + 1),
                                   KOFF[b] + P * t:KOFF[b] + P * (t + 1)],
                                qt[DH * g:DH * (g + 1),
                                   QOFF[b] + c0:QOFF[b] + c0 + n],
                                start=True, stop=True)
                    for g in range(2):
                        nc.scalar.activation(
                            out=ats[g][:, t0:t0 + gn, :n],
                            in_=pq[g][:, 0:gn, :n],
                            func=Exp, scale=0.125)
                    upto = (L * (gi + 1)) // len(groups)
                    while done < upto:
                        side[done][1]()
                        done += 1
                while done < L:
                    side[done][1]()
                    done += 1
                pendq.append(pv_pieces(b, c0, n, ats, tail))

            # ---- pass 1: build the chunk schedule (no emission) ----
            # carry_kt = batch's remaining kT chunks: chunk 0's later QK
            # groups need them, so they lead chunk 0's side.  carry_v = its
            # v pieces: must emit before PV of chunk 0 -> spread over
            # chunks 0 and 1.  q-proj pieces run two chunks ahead of their
            # QK consumer.  filler = next batch's k/q0 prefetch.
            sched = []
            carry_kt = prelude_rest(BORD[0])
            carry_v = prelude_b(BORD[0])
            for bi, b in enumerate(BORD):
                nxt = BORD[bi + 1] if bi + 1 < NB else None
                kt0_, v0_ = carry_kt, carry_v
                filler = (prelude_a(nxt) + prelude_rest(nxt)) \
                    if nxt is not None else []
                carry_kt = []
                carry_v = prelude_b(nxt) if nxt is not None else []
                qr = QP[b][1:]
                ch = chl[b]
                vh = (len(v0_) + 1) // 2 if len(ch) > 1 else len(v0_)
                fdone = 0
                for ci, (c0, n) in enumerate(ch):
                    take = (len(filler) * (ci + 1)) // len(ch) - fdone
                    if ci == 0:
                        qf = qr[0:2]
                    else:
                        qf = qr[ci + 1:ci + 2]
                    fresh = (kt0_ + v0_[:vh] if ci == 0 else []) \
                        + (v0_[vh:] if ci == 1 else []) \
                        + qf \
                        + filler[fdone:fdone + take]
                    fdone += take
                    sched.append((b, c0, n, fresh,
                                  bi == NB - 1 and ci >= len(ch) - 2))

            # ---- input stream in EXACT mm-consumption order ----
            srcs = {'k': xk_r, 'q': xq_r, 'v': xv_r}
            offs = {'k': KOFF, 'q': QOFF, 'v': KOFF}
            for key, _go in prelude_a(BORD[0]):
                stream_idx[key] = len(dma_stream)
                dma_stream.append(_mk_dma(
                    key, srcs[key[0]], offs[key[0]][key[1]], key[2],
                    chunk_n[key]))
            for _b, _c0, _n, fresh, _tl in sched:
                for key, _go in fresh:
                    if key is None or key in stream_idx:
                        continue
                    stream_idx[key] = len(dma_stream)
                    dma_stream.append(_mk_dma(
                        key, srcs[key[0]], offs[key[0]][key[1]], key[2],
                        chunk_n[key]))
            pump_dma()           # emit the initial XS_DEPTH transfers

            # ---- pass 2: emission ----
            for _key, go in prelude_a(BORD[0]):
                go()
            for b, c0, n, fresh, tail in sched:
                emit_chunk(b, c0, n, fresh, tail)
            for lst in pendq:
                for _key, go in lst:
                    go()
            pendq.clear()

    nc.compile()
    return nc


def _get_program(SQT, SKT):
    key = (tuple(SQT), tuple(SKT))
    if key not in _PROG_CACHE:
        _PROG_CACHE[key] = _build_program(key[0], key[1])
    return _PROG_CACHE[key]


def _prep_inputs(Q_seq, K_seq, V_seq, WQ, WK, WV, Q_len, V_len):
    """Host-side shared prep: per-batch transposed bf16 activations and
    masks, concatenated along seq; returns (SQT, SKT, shared dict)."""
    BF = ml_dtypes.bfloat16
    B = Q_seq.shape[0]
    SQT = [max(1, math.ceil(int(Q_len[b]) / P)) for b in range(B)]
    SKT = [max(1, math.ceil(int(V_len[b]) / P)) for b in range(B)]
    SQ = [t * P for t in SQT]
    SK = [t * P for t in SKT]

    xq = np.concatenate(
        [np.ascontiguousarray(Q_seq[b, :SQ[b]].T) for b in range(B)],
        axis=1).astype(BF)
    xk = np.concatenate(
        [np.ascontiguousarray(K_seq[b, :SK[b]].T) for b in range(B)],
        axis=1).astype(BF)
    mks = [(np.arange(SK[b]) < int(V_len[b])) for b in range(B)]
    xv = np.concatenate(
        [np.ascontiguousarray((V_seq[b, :SK[b]] * mks[b][:, None]).T)
         for b in range(B)], axis=1).astype(BF)
    # mask laid out partition-major [128, SKTtot, 2] so the DMA moves
    # contiguous per-partition lines
    mkcat = np.concatenate(mks)                       # [SKtot]
    mk2 = np.repeat(
        mkcat.reshape(-1, P).T[:, :, None], 2, axis=2).astype(BF)
    return SQT, SKT, {"xq": xq, "xk": xk, "xv": xv, "mk2": mk2}


def kernel(Q_seq, K_seq, V_seq, WQ, WK, WV, Q_len, V_len):
    global _last_nc, _last_in_maps
    _ensure_paths()
    from concourse.bass_utils import run_bass_kernel_spmd

    Q_seq = np.asarray(Q_seq, dtype=np.float32)
    K_seq = np.asarray(K_seq, dtype=np.float32)
    V_seq = np.asarray(V_seq, dtype=np.float32)
    WQ = np.asarray(WQ, dtype=np.float32)
    WK = np.asarray(WK, dtype=np.float32)
    WV = np.asarray(WV, dtype=np.float32)
    Q_len = np.asarray(Q_len).reshape(-1)
    V_len = np.asarray(V_len).reshape(-1)

    B, S, _ = Q_seq.shape
    BF = ml_dtypes.bfloat16

    SQT, SKT, shared = _prep_inputs(Q_seq, K_seq, V_seq, WQ, WK, WV,
                                    Q_len, V_len)
    SQ = [t * P for t in SQT]
    QOFF = [sum(SQ[:b]) for b in range(B)]

    nc = _get_program(SQT, SKT)

    in_maps = []
    for c in range(NCORES):
        m = dict(shared)
        m["wq"] = np.ascontiguousarray(WQ[:, P * c:P * (c + 1)]).astype(BF)
        m["wk"] = np.ascontiguousarray(WK[:, P * c:P * (c + 1)]).astype(BF)
        m["wv"] = np.ascontiguousarray(WV[:, P * c:P * (c + 1)]).astype(BF)
        in_maps.append(m)

    res = run_bass_kernel_spmd(nc, in_maps, core_ids=list(range(NCORES)))
    _last_nc, _last_in_maps = nc, in_maps

    H = 16
    full = np.zeros((B, S, H * DH), dtype=np.float32)
    for c in range(NCORES):
        o = np.asarray(res.results[c]["out"], dtype=np.float32)  # [2,65,SQtot]
        for g in range(2):
            h = 2 * c + g
            num = o[g, :DH]                # [64, SQtot]
            den = o[g, DH:DH + 1]          # [1, SQtot]
            ot = num / den
            for b in range(B):
                ql = int(Q_len[b])
                sl = ot[:, QOFF[b]:QOFF[b] + SQ[b]]
                full[b, :SQ[b], DH * h:DH * (h + 1)] = sl.T
                full[b, ql:, DH * h:DH * (h + 1)] = 0.0
    return full



# revision 37
# speedup vs baseline: 1.2240x; 1.0009x over previous
"""Multi-head attention (B=4, S=2048, D=1024, H=16, Dh=64) on 8 trn2 NeuronCores.

Sharding: core c -> heads (2c, 2c+1) of ALL 4 batches.  Every batch has 16
heads, so each core gets exactly 2 heads x 4 batches and per-core attention
work is Sum_b SQT_b*SKT_b score tiles -- perfectly balanced across cores
regardless of the per-batch sequence lengths (the old batch-sharded layout
made the largest-batch core ~2.1x slower than the mean).

Per core (2 heads, head A on partitions 0:64, head B on 64:128):
  - Host pre-transposes X per batch (D-major) in bf16 and concatenates the
    batches along seq: xq [D, SQtot], xk/xv [D, SKtot] (V rows >= V_len are
    zeroed on host).
  - Projections: qT/kT in [dh, seq] orientation, v in natural [seq, dh]
    orientation with a mask column appended per head (denominator trick).
  - QK computes scoresT[sk, sq] with K=64 contraction, the two heads issued
    back-to-back to complementary row groups (tile_position (0,0)/(64,0)) so
    they run concurrently in the PE array -- 2x QK throughput vs zero-padding
    the contraction to 128.
  - exp on ScalarE in groups of up to 3 sk-tiles (one 3-bank PSUM tile per
    group) to amortize the ~293ns fixed ACTIVATE overhead.
  - PV accumulates oT[65, sq] per head (row 64 = softmax denominator via the
    mask column).  NO on-device transpose or normalization: the kernel ships
    oT + denominator to DRAM and the HOST does o = (num/den).T and the
    Q_len row masking during unsharding.  This removes the fp32 PE-transpose
    matmuls (~80us of PE time in the old kernel) entirely.
  - Emission is software-pipelined: the next batch's projection pieces and
    the previous chunk's PV pieces are interleaved between QK groups to keep
    the in-order PE queue dense while ScalarE (the attention-phase
    bottleneck) drains the exp queue.

The program is compiled for the runtime tile counts (SQT_b, SKT_b) =
ceil(len/128) per batch (shared SPMD program across the 8 cores).
"""

import math

import numpy as np
import ml_dtypes


def _ensure_paths():
    import sys
    try:
        import concourse  # noqa: F401
        return
    except ImportError:
        pass
    for p in ("/opt/trn_rl_repo", "/root/.axon_site/_ro/trn_rl_repo"):
        if p not in sys.path:
            sys.path.insert(0, p)
    import concourse  # noqa: F401


P = 128          # SBUF partitions
D = 1024         # model dim
DH = 64          # head dim
KT = D // P      # contraction tiles for projections
GN = 3           # sk-tiles per exp group (3 PSUM banks)
NB = 4           # batches
NCORES = 8

_PROG_CACHE = {}

# exposed for test.py profiling reruns
_last_nc = None
_last_in_maps = None


def _chunks(total, sz=512):
    out = []
    o = 0
    while o < total:
        n = min(sz, total - o)
        out.append((o, n))
        o += n
    return out


def _build_program(SQT, SKT):
    """Build + bacc-compile the shared SPMD program for given per-batch tile
    counts (SQT, SKT are 4-tuples)."""
    _ensure_paths()
    import concourse.bass as bass  # noqa: F401
    import concourse.tile as tile
    from concourse import bacc, mybir

    BF = mybir.dt.bfloat16
    F32 = mybir.dt.float32
    Exp = mybir.ActivationFunctionType.Exp

    SQ = [t * P for t in SQT]
    SK = [t * P for t in SKT]
    SQtot = sum(SQ)
    SKtot = sum(SK)
    QOFF = [sum(SQ[:b]) for b in range(NB)]
    KOFF = [sum(SK[:b]) for b in range(NB)]
    TOFF = [sum(SKT[:b]) for b in range(NB)]
    SKTtot = sum(SKT)
    ATM = max(SKT)

    nc = bacc.Bacc("TRN2", target_bir_lowering=False, debug=False,
                   num_devices=NCORES)

    xq = nc.dram_tensor("xq", [D, SQtot], BF, kind="ExternalInput").ap()
    xk = nc.dram_tensor("xk", [D, SKtot], BF, kind="ExternalInput").ap()
    xv = nc.dram_tensor("xv", [D, SKtot], BF, kind="ExternalInput").ap()
    wq = nc.dram_tensor("wq", [D, P], BF, kind="ExternalInput").ap()
    wk = nc.dram_tensor("wk", [D, P], BF, kind="ExternalInput").ap()
    wv = nc.dram_tensor("wv", [D, P], BF, kind="ExternalInput").ap()
    mk2 = nc.dram_tensor("mk2", [P, SKTtot, 2], BF, kind="ExternalInput").ap()
    # bf16 output: halves the output DMA and the tail flush; the host
    # division num/den happens in fp32 so only ~0.3% quantization noise
    # is added on top of the bf16 matmul pipeline (gate is 2e-2)
    out = nc.dram_tensor("out", [2, DH + 1, SQtot], BF,
                         kind="ExternalOutput").ap()

    xq_r = xq.rearrange("(k p) s -> p k s", p=P)
    xk_r = xk.rearrange("(k p) s -> p k s", p=P)
    xv_r = xv.rearrange("(k p) s -> p k s", p=P)

    VW = 2 * (DH + 1)        # 130: [A num 64 | A mask | B num 64 | B mask]

    with tile.TileContext(nc) as tc:
        with tc.tile_pool(name="const", bufs=1) as const, \
             tc.tile_pool(name="persist", bufs=1) as persist, \
             tc.tile_pool(name="xs", bufs=10) as xs, \
             tc.tile_pool(name="atp", bufs=3) as atp, \
             tc.tile_pool(name="otp", bufs=2) as otp, \
             tc.tile_pool(name="psq", bufs=2, space="PSUM") as psq, \
             tc.tile_pool(name="pss", bufs=2, space="PSUM") as pss:

            wq_sb = const.tile([P, KT, P], BF, tag="wq")
            wk_sb = const.tile([P, KT, P], BF, tag="wk")
            wv_sb = const.tile([P, KT, P], BF, tag="wv")
            mask_sb = const.tile([P, SKTtot, 2], BF, tag="mk")
            # warm-up tile memset FIRST on the gpsimd engine (it is idle and
            # ready before Vector; and it must precede the w DMAs below so
            # the PE warm-up isn't queued behind their descriptor time)
            warm = const.tile([P, 512], BF, tag="warm")
            nc.gpsimd.memset(warm, 0.0)
            # wk on the sync queue ahead of the x stream (first QK group
            # needs it); wq/wv/mask via gpsimd so xk0 starts 0.8us earlier
            nc.sync.dma_start(out=wk_sb, in_=wk.rearrange("(k p) e -> p k e", p=P))
            nc.gpsimd.dma_start(out=wq_sb, in_=wq.rearrange("(k p) e -> p k e", p=P))
            nc.gpsimd.dma_start(out=wv_sb, in_=wv.rearrange("(k p) e -> p k e", p=P))
            nc.gpsimd.dma_start(out=mask_sb, in_=mk2)

            qt = persist.tile([P, SQtot], BF, tag="qt")
            kt = persist.tile([P, SKtot], BF, tag="kt")
            v_sb = persist.tile([P, SKTtot, VW], BF, tag="v")
            # the mask (denominator) columns of v_sb are constant: write
            # them all with ONE strided copy instead of 2 tiny DVE copies
            # per sk-subtile in the hot path
            nc.vector.tensor_copy(
                out=v_sb.rearrange("p t (g c) -> p t g c", c=DH + 1)[:, :, :, DH],
                in_=mask_sb)

            # PE warm-up: dummy matmuls with no input dependencies.
            # They run during the initial input-DMA wait and flip the HAM
            # clock gate to 8/8 so the first real matmuls run at 2.4 GHz.
            # All 20 target ONE psq-pool tile: rotating the pss pool here
            # would make the first projections wait on warm-up completions.
            wp = psq.tile([P, GN, 512], F32, tag="qk", name="warm_ps")
            for w in range(20):
                nc.tensor.matmul(wp[:, 0, :], warm[:, 0:P], warm,
                                 start=True, stop=True)

            # process batches LARGEST-first: the big batch's long exp phase
            # covers the later batches' input DMA, and only kT + q0 of the
            # first batch gate the start of attention
            BORD = sorted(range(NB), key=lambda b: -SQT[b] * SKT[b])

            # attention / q-proj chunk lists: a small first chunk for the
            # first batch (less DMA before the first exp) and a small final
            # chunk for the last batch (shorter PV tail)
            chl = {b: _chunks(SQ[b]) for b in range(NB)}
            kchl = {b: _chunks(SK[b]) for b in range(NB)}
            if SQ[BORD[0]] > 512:
                chl[BORD[0]] = [(0, 256)] + \
                    [(256 + o, n) for o, n in _chunks(SQ[BORD[0]] - 256)]
            if SK[BORD[0]] > 512:
                kchl[BORD[0]] = [(0, 384)] + \
                    [(384 + o, n) for o, n in _chunks(SK[BORD[0]] - 384)]
            _lo, _ln = chl[BORD[-1]][-1]
            if _ln > 128:
                chl[BORD[-1]] = chl[BORD[-1]][:-1] + \
                    [(_lo, _ln - 128), (_lo + _ln - 128, 128)]

            # ---------------- input-DMA pump + mm pieces ----------------
            # The x-input DMAs are DECOUPLED from the matmul pieces: all
            # transfers are emitted in need-order on the ONE sync queue,
            # ~XS_DEPTH pieces ahead of their consumers (the xs ring
            # throttles at runtime).  The queue then never idles, which
            # matters because the input stream (~26 MB at ~330 GB/s) has
            # almost no slack against the compute schedule.
            XS_DEPTH = 10
            dma_stream = []      # need-ordered [(key, emit_fn)]
            dma_emitted = {}     # key -> xt tile (set when DMA emitted)
            mm_done_idx = set()  # stream indices whose mm part has run
            pump_state = [0]     # next stream index to emit

            def _mk_dma(key, src_r, off, c0, n):
                def emit():
                    if key in dma_emitted:
                        return
                    xt = xs.tile([P, KT, 512], BF, tag="x")
                    nc.sync.dma_start(out=xt[:, :, :n],
                                      in_=src_r[:, :, off + c0:off + c0 + n])
                    dma_emitted[key] = xt
                return (key, emit)

            def pump_dma():
                """Emit pending input DMAs: stream index i may be emitted
                once the reader of ring slot i-XS_DEPTH has been emitted
                (pool WAR dep must point backward in program order)."""
                while pump_state[0] < len(dma_stream):
                    i = pump_state[0]
                    if i >= XS_DEPTH and i - XS_DEPTH not in mm_done_idx:
                        return
                    dma_stream[i][1]()
                    pump_state[0] = i + 1

            stream_idx = {}      # key -> position in dma_stream

            def q_proj_piece(b, c0, n, dst, w_sb, off, kind):
                key = (kind, b, c0)

                def go():
                    if key not in dma_emitted:       # safety: force-emit
                        dma_stream[stream_idx[key]][1]()
                    xt = dma_emitted[key]
                    ps = pss.tile([P, 512], F32, tag="acc")
                    for k in range(KT):
                        nc.tensor.matmul(ps[:, :n], w_sb[:, k, :], xt[:, k, :n],
                                         start=(k == 0), stop=(k == KT - 1))
                    nc.vector.tensor_copy(out=dst[:, off + c0:off + c0 + n],
                                          in_=ps[:, :n])
                    mm_done_idx.add(stream_idx[key])
                    pump_dma()
                return (key, go)

            def v_proj_piece(b, c0, n):
                key = ('v', b, c0)

                def go():
                    if key not in dma_emitted:       # safety: force-emit
                        dma_stream[stream_idx[key]][1]()
                    xt = dma_emitted[key]
                    nt = n // P
                    ta = TOFF[b] + c0 // P
                    for st in range(nt):
                        ps = pss.tile([P, 512], F32, tag="acc")
                        for k in range(KT):
                            nc.tensor.matmul(
                                ps[:, :P], xt[:, k, P * st:P * (st + 1)],
                                wv_sb[:, k, :],
                                start=(k == 0), stop=(k == KT - 1))
                        vt = v_sb[:, ta + st, 0:VW].rearrange(
                            "p (g c) -> p g c", c=DH + 1)
                        nc.vector.tensor_copy(
                            out=vt[:, :, 0:DH],
                            in_=ps[:, :P].rearrange("p (g c) -> p g c", c=DH))
                    mm_done_idx.add(stream_idx[key])
                    pump_dma()
                return (key, go)

            # per-batch piece lists, created ONCE (each emits exactly once)
            chunk_n = {}
            QP, KP, VP = {}, {}, {}
            for b in range(NB):
                QP[b] = [q_proj_piece(b, c0, n, qt, wq_sb, QOFF[b], 'q')
                         for c0, n in chl[b]]
                KP[b] = [q_proj_piece(b, c0, n, kt, wk_sb, KOFF[b], 'k')
                         for c0, n in kchl[b]]
                VP[b] = [v_proj_piece(b, c0, n) for c0, n in _chunks(SK[b])]
                for c0, n in chl[b]:
                    chunk_n[('q', b, c0)] = n
                for c0, n in kchl[b]:
                    chunk_n[('k', b, c0)] = n
                for c0, n in _chunks(SK[b]):
                    chunk_n[('v', b, c0)] = n

            def prelude_a(b):
                """What batch b's first QK group needs: first kT chunk + the
                first q chunk."""
                return [KP[b][0], QP[b][0]]

            def prelude_rest(b):
                return KP[b][1:]

            def prelude_b(b):
                """v projection -- only needed by PV, one chunk after QK."""
                return VP[b]

            # ---------------- attention ----------------
            def pv_pieces(b, c0, n, ats, tail_dma=False):
                """PV + evac + output-DMA closures for one finished chunk."""
                pieces = []

                def mk_pv(g, t0, t1, po_box):
                    def go():
                        if t0 == 0:
                            po_box[0] = pss.tile([P, 512], F32, tag="acc",
                                                 name=f"po_{b}_{c0}_{g}")
                        po = po_box[0]
                        for t in range(t0, t1):
                            # lhsT is only the 65 live columns: LDWEIGHTS
                            # time scales with column count
                            nc.tensor.matmul(
                                po[0:DH + 1, :n],
                                v_sb[:, TOFF[b] + t, (DH + 1) * g:
                                     (DH + 1) * (g + 1)],
                                ats[g][:, t, :n],
                                start=(t == 0), stop=(t == SKT[b] - 1))
                        if t1 == SKT[b]:
                            ot = otp.tile([DH + 1, 512], BF, tag="ot",
                                          name=f"ot_{b}_{c0}_{g}")
                            nc.vector.tensor_copy(out=ot[:, :n],
                                                  in_=po[0:DH + 1, :n])
                            # output DMA on the (idle) GpSimd queue so it
                            # never blocks input prefetch on the sync queue;
                            # the tail chunks go via the scalar queue so the
                            # final flush drains two queues in parallel
                            eng = nc.scalar if tail_dma else nc.gpsimd
                            eng.dma_start(
                                out=out[g, :, QOFF[b] + c0:QOFF[b] + c0 + n],
                                in_=ot[:, :n])
                    return go

                for g in range(2):
                    box = [None]
                    if SKT[b] > 1:
                        half = (SKT[b] + 1) // 2
                        pieces.append((None, mk_pv(g, 0, half, box)))
                        pieces.append((None, mk_pv(g, half, SKT[b], box)))
                    else:
                        pieces.append((None, mk_pv(g, 0, SKT[b], box)))
                return pieces

            # PV pieces of chunk c are emitted two chunks later (the at
            # ring is 3 deep), so side-work DMA has two chunks of exp time
            # to stream in before it can stall the PE queue
            pendq = []         # FIFO of per-chunk PV piece lists

            def emit_chunk(b, c0, n, fresh, tail=False, pop_n=1):
                """Emit one attention chunk; `fresh` (proj pieces) and the
                due PV pieces are interleaved between QK groups."""
                ats = (atp.tile([P, ATM, 512], BF, tag="ata",
                                name=f"ata_{b}_{c0}"),
                       atp.tile([P, ATM, 512], BF, tag="atb",
                                name=f"atb_{b}_{c0}"))
                groups = [(t0, min(GN, SKT[b] - t0))
                          for t0 in range(0, SKT[b], GN)]
                side = list(fresh)
                for _ in range(pop_n):
                    if pendq:
                        side = side + pendq.pop(0)
                L = len(side)
                done = 0
                for gi, (t0, gn) in enumerate(groups):
                    pq = [psq.tile([P, GN, 512], F32, tag="qk",
                                   name=f"qk_{b}_{c0}_{t0}_{g}")
                          for g in range(2)]
                    for j in range(gn):
                        t = t0 + j
                        for g in range(2):
                            nc.tensor.matmul(
                                pq[g][:, j, :n],
                                kt[DH * g:DH * (g + 1),
                                   KOFF[b] + P * t:KOFF[b] + P * (t + 1)],
                                qt[DH * g:DH * (g + 1),
                                   QOFF[b] + c0:QOFF[b] + c0 + n],
                                start=True, stop=True)
                    for g in range(2):
                        nc.scalar.activation(
                            out=ats[g][:, t0:t0 + gn, :n],
                            in_=pq[g][:, 0:gn, :n],
                            func=Exp, scale=0.125)
                    upto = (L * (gi + 1)) // len(groups)
                    while done < upto:
                        side[done][1]()
                        done += 1
                while done < L:
                    side[done][1]()
                    done += 1
                pendq.append(pv_pieces(b, c0, n, ats, tail))

            # ---- pass 1: build the chunk schedule (no emission) ----
            # carry_kt = batch's remaining kT chunks: chunk 0's later QK
            # groups need them, so they lead chunk 0's side.  carry_v = its
            # v pieces: must emit before PV of chunk 0 -> spread over
            # chunks 0 and 1.  q-proj pieces run two chunks ahead of their
            # QK consumer.  filler = next batch's k/q0 prefetch.
            sched = []
            carry_kt = prelude_rest(BORD[0])
            carry_v = prelude_b(BORD[0])
            for bi, b in enumerate(BORD):
                nxt = BORD[bi + 1] if bi + 1 < NB else None
                kt0_, v0_ = carry_kt, carry_v
                filler = (prelude_a(nxt) + prelude_rest(nxt)) \
                    if nxt is not None else []
                carry_kt = []
                carry_v = prelude_b(nxt) if nxt is not None else []
                qr = QP[b][1:]
                ch = chl[b]
                vh = (len(v0_) + 1) // 2 if len(ch) > 1 else len(v0_)
                # first batch: PV(c0) deferred to c2 (pend ring 2) so the v
                # transfers queue AFTER k/q on the wire-critical early phase;
                # its v mm pieces sit in chunks 1-2.  Later batches keep the
                # ring at depth 2; the last batch drains it back (pop 2).
                vc0, vc1 = (1, 2) if (bi == 0 and len(ch) > 2) else (0, 1)
                fdone = 0
                for ci, (c0, n) in enumerate(ch):
                    # back-loaded filler: the next batch's k/q0 prefetch
                    # runs in THIS batch's later chunks, freeing the wire
                    # for this batch's own v/q early on
                    tgt = (len(filler) * max(0, 2 * (ci + 1) - len(ch))) \
                        // len(ch)
                    take = tgt - fdone
                    if ci == 0:
                        qf = qr[0:2]
                    else:
                        qf = qr[ci + 1:ci + 2]
                    fresh = (kt0_ if ci == 0 else []) \
                        + (v0_[:vh] if ci == vc0 else []) \
                        + (v0_[vh:] if ci == vc1 else []) \
                        + qf \
                        + filler[fdone:fdone + take]
                    fdone += take
                    if bi == 0 and ci == 1:
                        pop_n = 0
                    elif bi == NB - 1 and ci <= 1:
                        pop_n = 2
                    else:
                        pop_n = 1
                    sched.append((b, c0, n, fresh, pop_n,
                                  bi == NB - 1 and ci >= len(ch) - 2))

            # ---- input stream in EXACT mm-consumption order ----
            srcs = {'k': xk_r, 'q': xq_r, 'v': xv_r}
            offs = {'k': KOFF, 'q': QOFF, 'v': KOFF}
            for key, _go in prelude_a(BORD[0]):
                stream_idx[key] = len(dma_stream)
                dma_stream.append(_mk_dma(
                    key, srcs[key[0]], offs[key[0]][key[1]], key[2],
                    chunk_n[key]))
            for _b, _c0, _n, fresh, _pn, _tl in sched:
                for key, _go in fresh:
                    if key is None or key in stream_idx:
                        continue
                    stream_idx[key] = len(dma_stream)
                    dma_stream.append(_mk_dma(
                        key, srcs[key[0]], offs[key[0]][key[1]], key[2],
                        chunk_n[key]))
            pump_dma()           # emit the initial XS_DEPTH transfers

            # ---- pass 2: emission ----
            for _key, go in prelude_a(BORD[0]):
                go()
            for b, c0, n, fresh, pop_n, tail in sched:
                emit_chunk(b, c0, n, fresh, tail, pop_n)
            for lst in pendq:
                for _key, go in lst:
                    go()
            pendq.clear()

    nc.compile()
    return nc


def _get_program(SQT, SKT):
    key = (tuple(SQT), tuple(SKT))
    if key not in _PROG_CACHE:
        _PROG_CACHE[key] = _build_program(key[0], key[1])
    return _PROG_CACHE[key]


def _prep_inputs(Q_seq, K_seq, V_seq, WQ, WK, WV, Q_len, V_len):
    """Host-side shared prep: per-batch transposed bf16 activations and
    masks, concatenated along seq; returns (SQT, SKT, shared dict)."""
    BF = ml_dtypes.bfloat16
    B = Q_seq.shape[0]
    SQT = [max(1, math.ceil(int(Q_len[b]) / P)) for b in range(B)]
    SKT = [max(1, math.ceil(int(V_len[b]) / P)) for b in range(B)]
    SQ = [t * P for t in SQT]
    SK = [t * P for t in SKT]

    xq = np.concatenate(
        [np.ascontiguousarray(Q_seq[b, :SQ[b]].T) for b in range(B)],
        axis=1).astype(BF)
    xk = np.concatenate(
        [np.ascontiguousarray(K_seq[b, :SK[b]].T) for b in range(B)],
        axis=1).astype(BF)
    mks = [(np.arange(SK[b]) < int(V_len[b])) for b in range(B)]
    xv = np.concatenate(
        [np.ascontiguousarray((V_seq[b, :SK[b]] * mks[b][:, None]).T)
         for b in range(B)], axis=1).astype(BF)
    # mask laid out partition-major [128, SKTtot, 2] so the DMA moves
    # contiguous per-partition lines
    mkcat = np.concatenate(mks)                       # [SKtot]
    mk2 = np.repeat(
        mkcat.reshape(-1, P).T[:, :, None], 2, axis=2).astype(BF)
    return SQT, SKT, {"xq": xq, "xk": xk, "xv": xv, "mk2": mk2}


def kernel(Q_seq, K_seq, V_seq, WQ, WK, WV, Q_len, V_len):
    global _last_nc, _last_in_maps
    _ensure_paths()
    from concourse.bass_utils import run_bass_kernel_spmd

    Q_seq = np.asarray(Q_seq, dtype=np.float32)
    K_seq = np.asarray(K_seq, dtype=np.float32)
    V_seq = np.asarray(V_seq, dtype=np.float32)
    WQ = np.asarray(WQ, dtype=np.float32)
    WK = np.asarray(WK, dtype=np.float32)
    WV = np.asarray(WV, dtype=np.float32)
    Q_len = np.asarray(Q_len).reshape(-1)
    V_len = np.asarray(V_len).reshape(-1)

    B, S, _ = Q_seq.shape
    BF = ml_dtypes.bfloat16

    SQT, SKT, shared = _prep_inputs(Q_seq, K_seq, V_seq, WQ, WK, WV,
                                    Q_len, V_len)
    SQ = [t * P for t in SQT]
    QOFF = [sum(SQ[:b]) for b in range(B)]

    nc = _get_program(SQT, SKT)

    in_maps = []
    for c in range(NCORES):
        m = dict(shared)
        m["wq"] = np.ascontiguousarray(WQ[:, P * c:P * (c + 1)]).astype(BF)
        m["wk"] = np.ascontiguousarray(WK[:, P * c:P * (c + 1)]).astype(BF)
        m["wv"] = np.ascontiguousarray(WV[:, P * c:P * (c + 1)]).astype(BF)
        in_maps.append(m)

    res = run_bass_kernel_spmd(nc, in_maps, core_ids=list(range(NCORES)))
    _last_nc, _last_in_maps = nc, in_maps

    H = 16
    full = np.zeros((B, S, H * DH), dtype=np.float32)
    for c in range(NCORES):
        o = np.asarray(res.results[c]["out"], dtype=np.float32)  # [2,65,SQtot]
        for g in range(2):
            h = 2 * c + g
            num = o[g, :DH]                # [64, SQtot]
            den = o[g, DH:DH + 1]          # [1, SQtot]
            ot = num / den
            for b in range(B):
                ql = int(Q_len[b])
                sl = ot[:, QOFF[b]:QOFF[b] + SQ[b]]
                full[b, :SQ[b], DH * h:DH * (h + 1)] = sl.T
                full[b, ql:, DH * h:DH * (h + 1)] = 0.0
    return full

